# revision 35
# baseline (speedup 1.0000x reference)
"""Multi-head attention (B=8, N=1024, C=768, H=12, D=64) on 8 TRN2
NeuronCores, data-parallel over batch. Self-contained: builds a Bass/Tile
kernel per core, runs SPMD via run_bass_kernel_spmd, returns full output.

Per-core dataflow:
  x[1024,768] -> xT[c,n] (PE transpose, f32)       wT=qkv_w.T[c,o], pwT=proj_w.T
  qkv matmuls in f32r (full-rate fp32-ish):
    q,k -> per-head bf16 tiles [128,1024], rows 0-63 = head data, 64-127 zero
           (K padded to 128: K=64 matmuls run at half rate on this PE)
    v   -> v_aug[n, 12*128] bf16: per head 64 v-cols + 64 ones-cols
  per head h (bf16 matmuls):
    scoresT[m,n] = k_pad[h][:,mslice].T @ q_pad[h]      (PSUM f32)
    expT[m,n] = exp(SCALE*scoresT)                      (ACT, bf16 out)
    pav[128,n] = v_aug[h-slice].T @ expT  — rows 0-63 attn@v, 64-127 the
           softmax denominator replicated 64x (ones-columns trick)
    bc = 1/pav[64:128]  (DVE reciprocal, full 64-partition op)
    attn_outT[c,n] = pav[0:64] * bc                     (f32r out)
  out[n,c'] = attn_outT.T @ pwT + bias (f32r) -> DMA out
"""
import sys

sys.path.insert(0, "/opt/trn_rl_repo")

from contextlib import ExitStack

import ml_dtypes
import numpy as np

import concourse.bass as bass
import concourse.mybir as mybir
import concourse.tile as tile
from concourse.bass_utils import run_bass_kernel_spmd
from concourse.masks import make_identity

DIM = 768
HEADS = 12
HD = 64
N = 1024
SCALE = HD ** -0.5
P = 128
NT = N // P          # 8 n-tiles
CT = DIM // P        # 6 c-tiles
F32 = mybir.dt.float32
F32R = mybir.dt.float32r
BF16 = mybir.dt.bfloat16
Exp = mybir.ActivationFunctionType.Exp
Ln = mybir.ActivationFunctionType.Ln

N_CORES = 8


def build_nc():
    nc = bass.Bass(trn_type="TRN2", target_bir_lowering=False, debug=False,
                   enable_asserts=False)
    x_d = nc.declare_dram_parameter("x", [N, DIM], F32, isOutput=False).ap()
    qkvwt_d = nc.declare_dram_parameter("qkv_wT", [DIM, 3 * DIM], BF16, isOutput=False).ap()
    qkvb_d = nc.declare_dram_parameter("qkv_b", [3 * DIM], F32, isOutput=False).ap()
    projwt_d = nc.declare_dram_parameter("proj_wT", [DIM, DIM], BF16, isOutput=False).ap()
    projb_d = nc.declare_dram_parameter("proj_b", [DIM], F32, isOutput=False).ap()
    out_d = nc.declare_dram_parameter("out", [N, DIM], F32, isOutput=True).ap()

    with tile.TileContext(nc) as tc, ExitStack() as top:
        const = top.enter_context(tc.tile_pool(name="const", bufs=1))
        identity = const.tile([P, P], F32)
        make_identity(nc, identity[:])
        identity_b = const.tile([P, P], BF16)
        nc.vector.tensor_copy(identity_b[:], identity[:])
        ones = const.tile([P, P], F32R)  # all-ones, f32r (rounded via copy below)

        bcol_qk = const.tile([P, 2 * CT], F32)  # column ot = qkv_b[ot*128:+128]
        nc.sync.dma_start(bcol_qk[:], qkvb_d[0:2 * DIM].rearrange("(o p) -> p o", p=P))

        # broadcast bias tiles for v and proj ([128, 768], same row repeated)
        vbias = const.tile([P, DIM], F32)
        pbias = const.tile([P, DIM], F32)
        with tc.tile_pool(name="brow_pool", bufs=1) as brow_pool, \
             tc.tile_pool(name="psum_bias", bufs=1, space="PSUM") as psum_bias:
            ones_f = brow_pool.tile([P, P], F32)
            nc.vector.memset(ones_f[:], 1.0)
            nc.vector.tensor_copy(ones[:], ones_f[:])
            b_row_f = brow_pool.tile([1, 3 * DIM], F32)
            nc.sync.dma_start(b_row_f[:], qkvb_d.unsqueeze(0))
            pb_row_f = brow_pool.tile([1, DIM], F32)
            nc.sync.dma_start(pb_row_f[:], projb_d.unsqueeze(0))
            b_row = brow_pool.tile([1, 3 * DIM], F32R)
            nc.vector.tensor_copy(b_row[:], b_row_f[:])
            pb_row = brow_pool.tile([1, DIM], F32R)
            nc.vector.tensor_copy(pb_row[:], pb_row_f[:])
            for dst, src_row, off in ((vbias, b_row, 2 * DIM), (pbias, pb_row, 0)):
                pt = psum_bias.tile([P, DIM], F32, tag="pbias", name="pbias")
                for o0, osz in ((0, 512), (512, 256)):
                    nc.tensor.matmul(pt[:, o0:o0 + osz], ones[0:1, :],
                                     src_row[0:1, off + o0:off + o0 + osz],
                                     start=True, stop=True)
                nc.vector.tensor_copy(dst[:], pt[:])

        # persistent activations: padded per-head q/k (bf16), interleaved v_aug
        qkvT = top.enter_context(tc.tile_pool(name="qkvT", bufs=1))
        q_pad = [qkvT.tile([P, N], BF16, tag=f"qp{h}", name=f"qp{h}") for h in range(HEADS)]
        k_pad = [qkvT.tile([P, N], BF16, tag=f"kp{h}", name=f"kp{h}") for h in range(HEADS)]
        v_aug = [qkvT.tile([P, HEADS * P], BF16, tag=f"v{i}", name=f"v{i}") for i in range(NT)]
        for h in range(HEADS):
            nc.gpsimd.memset(q_pad[h][HD:P, :], 0.0)
            nc.gpsimd.memset(k_pad[h][HD:P, :], 0.0)
        for nt in range(NT):
            # ones-columns 64:128 per head (denominator replication trick)
            va3 = v_aug[nt][:].rearrange("p (h e) -> p h e", e=P)
            nc.gpsimd.memset(va3[:, :, HD:P], 1.0)

        aoT_pool = top.enter_context(tc.tile_pool(name="aoT", bufs=1))
        attn_outT = [aoT_pool.tile([P, N], BF16, tag=f"aoT{i}", name=f"aoT{i}") for i in range(CT)]
        pw_pool = top.enter_context(tc.tile_pool(name="pwT", bufs=1))
        pwT = [pw_pool.tile([P, DIM], BF16, tag=f"pwT{i}", name=f"pwT{i}") for i in range(CT)]

        # ---- fused phase: loads/transposes + v + per-pair qk + attention ----
        with tc.tile_pool(name="xw", bufs=1) as xw, \
             tc.tile_pool(name="stage", bufs=2) as stage, \
             tc.tile_pool(name="expp", bufs=3) as expp, \
             tc.tile_pool(name="small", bufs=1) as small, \
             tc.tile_pool(name="outp", bufs=2) as outp, \
             tc.tile_pool(name="psum_t", bufs=1, space="PSUM") as psum_t, \
             tc.tile_pool(name="psum_big", bufs=2, space="PSUM") as psum_big, \
             tc.tile_pool(name="psum_s", bufs=3, space="PSUM") as psum_s_pool:
            xT = [xw.tile([P, N], BF16, tag=f"xT{i}", name=f"xT{i}") for i in range(CT)]
            wT = [xw.tile([P, 3 * DIM], BF16, tag=f"wT{i}", name=f"wT{i}") for i in range(CT)]

            def load_w(ct):
                nc.sync.dma_start(wT[ct][:], qkvwt_d[ct * P:(ct + 1) * P, :])

            def load_pw(ct):
                nc.sync.dma_start(pwT[ct][:], projwt_d[ct * P:(ct + 1) * P, :])

            def load_transpose(dram, row, dst_tiles, dst_col):
                st = stage.tile([P, DIM], F32, tag="stage", name="stage")
                nc.sync.dma_start(st[:], dram[row * P:(row + 1) * P, :])
                stb = stage.tile([P, DIM], BF16, tag="stageb", name="stageb")
                nc.vector.tensor_copy(stb[:], st[:])
                for ct in range(CT):
                    pt = psum_t.tile([P, P], BF16, tag="pt", name="pt")
                    nc.tensor.transpose(pt[:], stb[:, ct * P:(ct + 1) * P], identity_b[:])
                    nc.vector.tensor_copy(dst_tiles[ct][:, dst_col * P:(dst_col + 1) * P], pt[:])

            def emit_qk(t):
                ha, hb = 2 * t, 2 * t + 1
                for ot, pads in ((t, q_pad), (CT + t, k_pad)):
                    pq = psum_big.tile([P, N], F32, tag="pqk", name="pq")
                    for nch in range(2):
                        sl = slice(nch * 512, (nch + 1) * 512)
                        for ct in range(CT):
                            nc.tensor.matmul(
                                pq[:, sl],
                                wT[ct][:, ot * P:(ot + 1) * P],
                                xT[ct][:, sl],
                                start=(ct == 0), stop=(ct == CT - 1))
                        nc.vector.tensor_scalar_add(pads[ha][0:HD, sl], pq[0:HD, sl],
                                                    bcol_qk[0:HD, ot:ot + 1])
                        nc.vector.tensor_scalar_add(pads[hb][0:HD, sl], pq[HD:P, sl],
                                                    bcol_qk[HD:P, ot:ot + 1])

            def emit_scores_exp(h):
                expT = [expp.tile([P, N], BF16, tag=f"expT{mt}", name=f"expT{mt}_{h}")
                        for mt in range(NT)]
                for mt in range(NT):
                    for nch in range(2):
                        ps = psum_s_pool.tile([P, 512], F32, tag="ps", name="ps")
                        nc.tensor.matmul(
                            ps[:],
                            k_pad[h][:, mt * P:(mt + 1) * P],
                            q_pad[h][:, nch * 512:(nch + 1) * 512],
                            start=True, stop=True)
                        nc.scalar.activation(
                            expT[mt][:, nch * 512:(nch + 1) * 512],
                            ps[:], Exp, scale=SCALE)
                return expT

            def emit_av_norm(h, expT):
                t_i, t_off = h // 2, (h % 2) * HD
                pav = psum_big.tile([P, N], F32, tag="pqk", name="pav")
                for nch in range(2):
                    for mt in range(NT):
                        nc.tensor.matmul(
                            pav[:, nch * 512:(nch + 1) * 512],
                            v_aug[mt][:, h * P:(h + 1) * P],
                            expT[mt][:, nch * 512:(nch + 1) * 512],
                            start=(mt == 0), stop=(mt == NT - 1))
                # 1/D as exp(-ln D) on ACT
                lnd = small.tile([HD, N], F32, tag="lnd", name="lnd")
                nc.scalar.activation(lnd[:], pav[HD:P, :], Ln)
                nc.scalar.activation(lnd[:], lnd[:], Exp, scale=-1.0)
                nc.vector.tensor_mul(
                    attn_outT[t_i][t_off:t_off + HD, :], pav[0:HD, :], lnd[:])

            # pair-0 fast start: emit only x0-3-dependent chunks before
            # the x4-7 loads (emission order is program order in Tile)
            pq_q = psum_big.tile([P, N], F32, tag="pqk", name="pq0q")
            pq_k = psum_big.tile([P, N], F32, tag="pqk", name="pq0k")
            expT0 = [expp.tile([P, N], BF16, tag=f"expT{mt}", name=f"expT{mt}_0")
                     for mt in range(NT)]

            def qk0_chunk(nch):
                sl = slice(nch * 512, (nch + 1) * 512)
                for pq, ot, pads in ((pq_q, 0, q_pad), (pq_k, CT, k_pad)):
                    for ct in range(CT):
                        nc.tensor.matmul(
                            pq[:, sl], wT[ct][:, ot * P:(ot + 1) * P], xT[ct][:, sl],
                            start=(ct == 0), stop=(ct == CT - 1))
                    nc.vector.tensor_scalar_add(pads[0][0:HD, sl], pq[0:HD, sl],
                                                bcol_qk[0:HD, ot:ot + 1])
                    nc.vector.tensor_scalar_add(pads[1][0:HD, sl], pq[HD:P, sl],
                                                bcol_qk[HD:P, ot:ot + 1])

            def sc0_chunk(mt, nch):
                ps = psum_s_pool.tile([P, 512], F32, tag="ps", name="ps")
                nc.tensor.matmul(ps[:], k_pad[0][:, mt * P:(mt + 1) * P],
                                 q_pad[0][:, nch * 512:(nch + 1) * 512],
                                 start=True, stop=True)
                nc.scalar.activation(expT0[mt][:, nch * 512:(nch + 1) * 512],
                                     ps[:], Exp, scale=SCALE)

            for nt in range(4):
                load_transpose(x_d, nt, xT, nt)
            for ct in range(CT):
                load_w(ct)
            qk0_chunk(0)
            for mt in range(4):
                sc0_chunk(mt, 0)
            for nt in range(4, NT):
                load_transpose(x_d, nt, xT, nt)
            qk0_chunk(1)
            for mt in range(4):
                sc0_chunk(mt, 1)
            for mt in range(4, NT):
                for nch in range(2):
                    sc0_chunk(mt, nch)
            exps = {0: expT0, 1: emit_scores_exp(1)}
            emit_qk(1)
            exps[2] = emit_scores_exp(2)

            # v matmuls (bf16), interleaved 128-stride (64 v-cols + 64 ones)
            for nt in range(NT):
                pv = psum_big.tile([P, DIM], F32, tag="pqk", name="pv")
                for o0, osz in ((0, 512), (512, 256)):
                    for ct in range(CT):
                        nc.tensor.matmul(
                            pv[:, o0:o0 + osz],
                            xT[ct][:, nt * P:(nt + 1) * P],
                            wT[ct][:, 2 * DIM + o0:2 * DIM + o0 + osz],
                            start=(ct == 0), stop=(ct == CT - 1))
                va3 = v_aug[nt][:].rearrange("p (h e) -> p h e", e=P)
                for h0, hn, o0 in ((0, 8, 0), (8, 4, 512)):
                    nc.vector.tensor_add(
                        va3[:, h0:h0 + hn, 0:HD],
                        pv[:, o0:o0 + hn * HD].rearrange("p (h e) -> p h e", e=HD),
                        vbias[:, o0:o0 + hn * HD].rearrange("p (h e) -> p h e", e=HD))

            qk_done = 2
            for h in range(HEADS):
                emit_av_norm(h, exps.pop(h))
                nh = h + 3
                if nh < HEADS:
                    if qk_done <= nh // 2:
                        emit_qk(qk_done)
                        qk_done += 1
                    exps[nh] = emit_scores_exp(nh)

            # ---- proj (bf16) ----
            for ct2 in range(CT):
                load_pw(ct2)
            for nt in range(NT):
                po = psum_big.tile([P, DIM], F32, tag="pqk", name="po")
                for o0, osz in ((0, 512), (512, 256)):
                    for ct in range(CT):
                        nc.tensor.matmul(
                            po[:, o0:o0 + osz],
                            attn_outT[ct][:, nt * P:(nt + 1) * P],
                            pwT[ct][:, o0:o0 + osz],
                            start=(ct == 0), stop=(ct == CT - 1))
                ot_t = outp.tile([P, DIM], F32, tag="out", name="out")
                nc.vector.tensor_add(ot_t[:], po[:], pbias[:])
                nc.sync.dma_start(out_d[nt * P:(nt + 1) * P, :], ot_t[:])

    split_waits(nc)
    return nc


def split_waits(nc):
    """This walrus codegen supports one sync wait per instruction; move
    extra Tile-emitted waits onto EventSemaphore instructions inserted
    just before, in the same engine's program order."""
    n_split = 0
    for bb in nc.m.functions[0].blocks:
        insts = bb.instructions
        new_insts = []
        for inst in insts:
            si = inst.sync_info
            if si is not None and si.on_wait and len(si.on_wait) > 1:
                waits = list(si.on_wait)
                for w in waits[:-1]:
                    ev = mybir.InstEventSemaphore(name=f"{inst.name}-ws{n_split}")
                    ev.engine = inst.engine
                    ev.sync_info = mybir.SyncInfo(on_wait=[w], on_update=[])
                    new_insts.append(ev)
                    n_split += 1
                si.on_wait = [waits[-1]]
                inst.sync_info = si
            new_insts.append(inst)
        if len(new_insts) != len(insts):
            insts[:] = new_insts
    return n_split


_NC_CACHE = None


def get_nc():
    global _NC_CACHE
    if _NC_CACHE is None:
        _NC_CACHE = build_nc()
    return _NC_CACHE


def run(inputs, **kwargs):
    nc = get_nc()
    x = np.ascontiguousarray(inputs["x"], dtype=np.float32)
    shared = {
        "qkv_wT": np.ascontiguousarray(
            np.asarray(inputs["qkv_w"], dtype=np.float32).T).astype(ml_dtypes.bfloat16),
        "qkv_b": np.ascontiguousarray(inputs["qkv_b"], dtype=np.float32),
        "proj_wT": np.ascontiguousarray(
            np.asarray(inputs["proj_w"], dtype=np.float32).T).astype(ml_dtypes.bfloat16),
        "proj_b": np.ascontiguousarray(inputs["proj_b"], dtype=np.float32),
    }
    in_maps = [{"x": x[i], **shared} for i in range(N_CORES)]
    res = run_bass_kernel_spmd(nc, in_maps, core_ids=list(range(N_CORES)), **kwargs)
    out = np.stack([res.results[i]["out"] for i in range(N_CORES)], axis=0)
    return out, res


def kernel(x, qkv_w, qkv_b, proj_w, proj_b):
    out, _ = run({"x": x, "qkv_w": qkv_w, "qkv_b": qkv_b,
                  "proj_w": proj_w, "proj_b": proj_b})
    return out
